# revision 14
# baseline (speedup 1.0000x reference)
"""GridQuantizer VQ kernel for Trainium2 (8 NeuronCores, data-parallel over N).

The proto table is a separable uniform 128x128 meshgrid of per-dim midpoints
(identical in both dims for this problem: first=-127/128, step=2^-6), so
nearest-proto search collapses to per-dim nearest-midpoint rounding:

    t   = x*64 + 63.5              # grid coordinate relative to first midpoint
    v   = clamp(round(t), 0, 127)  # nearest midpoint index
    dfg = t - v                    # offset in grid units (error-free fp32)
    d2g = dfg0^2 + dfg1^2
    pos = v1*128 + v0
    mindist = sqrt(d2g) * step     # host; exact power-of-2 scaling

round() is the +-2^23 RNE trick with both clamps folded into the round's two
tensor_scalar ops: m = min(t + 2^23, 2^23+127) rounds AND clamps high while
still >= 2^23 (ulp=1, exact integers); v = max(m - 2^23, 0) clamps low,
catching both out-of-grid negatives and the sub-2^23 half-ulp quirk (for
t < 0 the sum sits below 2^23 where ulp=0.5, so m - 2^23 can be -0.5).
dfg uses the UNclamped t so out-of-grid points get their true distance to the
edge proto. All constants derive from the actual protos input and are
asserted against it at run time.

Per core: x shard [1024, 2] -> SBUF [128, 16] (point i = p*8+k at row p,
cols 2k/2k+1). The 7-op DVE chain writes interleaved (d2g, posf) pairs to one
[128, 16] output tile.

Schedule (raw engine streams, no Block):
  SP:   input DMA issued at t=0 (hoisted ahead of SP register init; the DMA
        has static APs and reads no SP registers), then the output DMA with
        the compute-done wait attached to the DMA instruction itself.
  DVE:  constants + chain; the input wait is attached to the first chain op
        so the sequencer pre-decodes the chain during the DMA window.
The Bass-constructor all-engine start barrier and const-AP memsets (dead code
here) are stripped at build time.  (A gpsimd kv_writeback prepare/trigger
output path would shave another ~1.3us of HWDGE+DGE latency off the tail,
but those custom Pool ISA instructions don't compile through the external
neuronxcc walrus backend this environment uses.)
"""

import numpy as np

N_CORES = 8
N = 8192
PTS = N // N_CORES      # 1024 points per core
P = 128                 # SBUF partitions
K = PTS // P            # 8 points per partition
COLS = 2 * K            # 16 interleaved values per partition
GRID = 128              # protos per dimension
OUT_SHAPE = (PTS, 2)    # row i: (d2_grid, posf)

M23 = 8388608.0         # 2^23
HI = M23 + float(GRID - 1)

FIRST = -0.9921875      # first midpoint, both dims
STEP = 0.015625         # 2/128, both dims
INV = 64.0              # 1/STEP
C1 = -FIRST * INV       # 63.5


def _build_program():
    import concourse.bass as bass
    from concourse import mybir

    f32 = mybir.dt.float32
    Alu = mybir.AluOpType

    # Suppress the constructor's all-engine start barrier and const-AP
    # memsets: dead weight on this kernel's critical path (the barrier's
    # straggler engine delays the input DMA by ~750ns).
    noop = lambda *a, **kw: None
    patched = [
        (bass.Bass, "all_engine_barrier", bass.Bass.all_engine_barrier),
        (bass.BassEitherVectorEngine, "memset", bass.BassEitherVectorEngine.memset),
    ]
    for cls, attr, _ in patched:
        setattr(cls, attr, noop)
    try:
        nc = bass.Bass(
            target_bir_lowering=False,
            monotonic_sem_count=0,
            detect_race_conditions=False,
        )
    finally:
        for cls, attr, orig in patched:
            setattr(cls, attr, orig)

    x = nc.dram_tensor("x", [PTS, 2], f32, kind="ExternalInput")
    out = nc.dram_tensor("out", list(OUT_SHAPE), f32, kind="ExternalOutput")

    ctx = nc.ctx
    in_sem = ctx.enter_context(nc.semaphore("in_sem"))
    cmp_sem = ctx.enter_context(nc.semaphore("cmp_sem"))
    out_sem = ctx.enter_context(nc.semaphore("out_sem"))

    xt = ctx.enter_context(nc.sbuf_tensor("xt", [P, COLS], f32))
    tt_ = ctx.enter_context(nc.sbuf_tensor("tbuf", [P, COLS], f32))
    mt = ctx.enter_context(nc.sbuf_tensor("mbuf", [P, COLS], f32))
    vt = ctx.enter_context(nc.sbuf_tensor("vbuf", [P, COLS], f32))
    ot = ctx.enter_context(nc.sbuf_tensor("ot", [P, COLS], f32))
    c_m23 = ctx.enter_context(nc.sbuf_tensor("c_m23", [P, 1], f32))
    c_hi = ctx.enter_context(nc.sbuf_tensor("c_hi", [P, 1], f32))
    c_zero = ctx.enter_context(nc.sbuf_tensor("c_zero", [P, 1], f32))

    # --- SP: input DMA, then output DMA gated on compute completion ---
    in_dma = nc.sync.dma_start(
        xt[:], x[:].rearrange("(p k) two -> p (k two)", p=P)
    ).then_inc(in_sem, 16)
    cmp_wait = nc.sync.wait_ge(cmp_sem, 1)
    out_dma = nc.sync.dma_start(
        out[:].rearrange("(p k) two -> p (k two)", p=P), ot[:]
    ).then_inc(out_sem, 16)
    nc.sync.wait_ge(out_sem, 16)

    # --- DVE: constants + 7-op chain ---
    vector = nc.vector
    vector.memset(c_m23[:], M23)
    vector.memset(c_hi[:], HI)
    vector.memset(c_zero[:], 0.0)
    in_wait = vector.wait_ge(in_sem, 16)
    # The DVE pipeline has NO reliable same-engine RAW interlock: a
    # dependent read issued right behind a write intermittently sees stale
    # data on real HW (observed: op4 reading pre-op3 vbuf; op5's self-mult
    # reading a stale Ptr-scalar giving dfg*2^23). Every dependent stage
    # must be separated by a drain. The two final STT ops are mutually
    # independent and share one drain.
    chain0 = vector.tensor_scalar(tt_[:], xt[:], INV, C1, Alu.mult, Alu.add)
    vector.drain()
    vector.tensor_scalar(mt[:], tt_[:], c_m23[:], c_hi[:], Alu.add, Alu.min)
    vector.drain()
    vector.tensor_scalar(vt[:], mt[:], c_m23[:], c_zero[:], Alu.subtract, Alu.max)
    vector.drain()
    vector.tensor_tensor(mt[:], tt_[:], vt[:], Alu.subtract)
    vector.drain()
    vector.tensor_tensor(tt_[:], mt[:], mt[:], Alu.mult)
    vector.drain()
    sq3 = tt_[:].rearrange("p (k two) -> p k two", two=2)
    o3 = ot[:].rearrange("p (k two) -> p k two", two=2)
    v3 = vt[:].rearrange("p (k two) -> p k two", two=2)
    # Note: plain TensorTensor/TensorReduce with strided [P,K,2]-view
    # operands mis-executes on real HW even when fenced — the
    # TensorScalarPtr scalar_tensor_tensor form works (verified).
    vector.scalar_tensor_tensor(
        o3[:, :, 0], sq3[:, :, 0], 1.0, sq3[:, :, 1], Alu.mult, Alu.add
    )
    vector.scalar_tensor_tensor(
        o3[:, :, 1], v3[:, :, 1], float(GRID), v3[:, :, 0], Alu.mult, Alu.add
    )
    # The completion signal rides a drain so all writes are committed
    # before the out-DMA reads ot.
    vector.drain().then_inc(cmp_sem, 1)

    # --- IR surgery: attach waits to their consumers, hoist the input DMA ---
    fn = nc.m.functions[0]
    bb = fn.blocks[0]
    insts = list(bb.instructions)

    def attach_wait(dst, src):
        w = list(src.ins.sync_info.on_wait)
        si = dst.ins.sync_info
        if si is None:
            dst.ins.sync_info = mybir.SyncInfo(on_wait=w, on_update=[])
        else:
            si.on_wait.extend(w)

    attach_wait(chain0, in_wait)
    attach_wait(out_dma, cmp_wait)

    dead = {in_wait.ins.name, cmp_wait.ins.name}
    insts = [i for i in insts if i.name not in dead]
    insts.remove(in_dma.ins)
    insts.insert(0, in_dma.ins)
    bb.instructions = insts

    nc.finalize()
    return nc


_CACHE = {}


def _get_program():
    if "nc" not in _CACHE:
        _CACHE["nc"] = _build_program()
    return _CACHE["nc"]


def _check_grid(protos):
    """The program hardcodes the grid; verify the input actually matches."""
    g = np.asarray(protos, dtype=np.float32)
    mids = np.arange(GRID, dtype=np.float64) * STEP + FIRST
    assert g.shape == (GRID * GRID, 2), g.shape
    assert np.array_equal(g[:GRID, 0].astype(np.float64), mids), "dim0 grid mismatch"
    assert np.array_equal(g[::GRID, 1].astype(np.float64), mids), "dim1 grid mismatch"


def kernel(x, protos):
    from concourse.bass_utils import run_bass_kernel_spmd

    x = np.ascontiguousarray(np.asarray(x, dtype=np.float32))
    _check_grid(protos)

    nc = _get_program()
    shards = np.split(x, N_CORES, axis=0)
    in_maps = [{"x": s} for s in shards]
    res = run_bass_kernel_spmd(nc, in_maps, core_ids=list(range(N_CORES)))

    d2 = np.empty(N, dtype=np.float32)
    posf = np.empty(N, dtype=np.float32)
    for c, r in enumerate(res.results):
        o = np.asarray(r["out"], dtype=np.float32).reshape(PTS, 2)
        d2[c * PTS : (c + 1) * PTS] = o[:, 0]
        posf[c * PTS : (c + 1) * PTS] = o[:, 1]

    mindist = np.sqrt(d2, dtype=np.float32) * np.float32(STEP)
    pos = posf.astype(np.int32)
    return mindist, pos


# revision 15
# speedup vs baseline: 1.0002x; 1.0002x over previous
"""GridQuantizer VQ kernel for Trainium2 (8 NeuronCores, data-parallel over N).

The proto table is a separable uniform 128x128 meshgrid of per-dim midpoints
(identical in both dims for this problem: first=-127/128, step=2^-6), so
nearest-proto search collapses to per-dim nearest-midpoint rounding:

    t   = x*64 + 63.5              # grid coordinate relative to first midpoint
    v   = clamp(round(t), 0, 127)  # nearest midpoint index
    dfg = t - v                    # offset in grid units (error-free fp32)
    d2g = dfg0^2 + dfg1^2
    pos = v1*128 + v0
    mindist = sqrt(d2g) * step     # host; exact power-of-2 scaling

round() is the +-2^23 RNE trick with both clamps folded into the round's two
tensor_scalar ops: m = min(t + 2^23, 2^23+127) rounds AND clamps high while
still >= 2^23 (ulp=1, exact integers); v = max(m - 2^23, 0) clamps low,
catching both out-of-grid negatives and the sub-2^23 half-ulp quirk (for
t < 0 the sum sits below 2^23 where ulp=0.5, so m - 2^23 can be -0.5).
dfg uses the UNclamped t so out-of-grid points get their true distance to the
edge proto. All constants derive from the actual protos input and are
asserted against it at run time.

Per core: x shard [1024, 2] -> SBUF [128, 16] (point i = p*8+k at row p,
cols 2k/2k+1). The 7-op DVE chain writes interleaved (d2g, posf) pairs to one
[128, 16] output tile.

Schedule (raw engine streams, no Block):
  SP:   input DMA issued at t=0 (hoisted ahead of SP register init; the DMA
        has static APs and reads no SP registers), then the output DMA with
        the compute-done wait attached to the DMA instruction itself.
  DVE:  constants + chain; the input wait is attached to the first chain op
        so the sequencer pre-decodes the chain during the DMA window.
The Bass-constructor all-engine start barrier and const-AP memsets (dead code
here) are stripped at build time.  (A gpsimd kv_writeback prepare/trigger
output path would shave another ~1.3us of HWDGE+DGE latency off the tail,
but those custom Pool ISA instructions don't compile through the external
neuronxcc walrus backend this environment uses.)
"""

import numpy as np

N_CORES = 8
N = 8192
PTS = N // N_CORES      # 1024 points per core
P = 128                 # SBUF partitions
K = PTS // P            # 8 points per partition
COLS = 2 * K            # 16 interleaved values per partition
GRID = 128              # protos per dimension
OUT_SHAPE = (PTS, 2)    # row i: (d2_grid, posf)

M23 = 8388608.0         # 2^23
HI = M23 + float(GRID - 1)

FIRST = -0.9921875      # first midpoint, both dims
STEP = 0.015625         # 2/128, both dims
INV = 64.0              # 1/STEP
C1 = -FIRST * INV       # 63.5


def _build_program():
    import concourse.bass as bass
    from concourse import mybir

    f32 = mybir.dt.float32
    Alu = mybir.AluOpType

    # Suppress the constructor's all-engine start barrier and const-AP
    # memsets: dead weight on this kernel's critical path (the barrier's
    # straggler engine delays the input DMA by ~750ns).
    noop = lambda *a, **kw: None
    patched = [
        (bass.Bass, "all_engine_barrier", bass.Bass.all_engine_barrier),
        (bass.BassEitherVectorEngine, "memset", bass.BassEitherVectorEngine.memset),
    ]
    for cls, attr, _ in patched:
        setattr(cls, attr, noop)
    try:
        nc = bass.Bass(
            target_bir_lowering=False,
            monotonic_sem_count=0,
            detect_race_conditions=False,
        )
    finally:
        for cls, attr, orig in patched:
            setattr(cls, attr, orig)

    x = nc.dram_tensor("x", [PTS, 2], f32, kind="ExternalInput")
    out = nc.dram_tensor("out", list(OUT_SHAPE), f32, kind="ExternalOutput")

    ctx = nc.ctx
    in_sem = ctx.enter_context(nc.semaphore("in_sem"))
    cmp_sem = ctx.enter_context(nc.semaphore("cmp_sem"))
    out_sem = ctx.enter_context(nc.semaphore("out_sem"))

    xt = ctx.enter_context(nc.sbuf_tensor("xt", [P, COLS], f32))
    tt_ = ctx.enter_context(nc.sbuf_tensor("tbuf", [P, COLS], f32))
    mt = ctx.enter_context(nc.sbuf_tensor("mbuf", [P, COLS], f32))
    vt = ctx.enter_context(nc.sbuf_tensor("vbuf", [P, COLS], f32))
    ot = ctx.enter_context(nc.sbuf_tensor("ot", [P, COLS], f32))
    c_m23 = ctx.enter_context(nc.sbuf_tensor("c_m23", [P, 1], f32))
    c_hi = ctx.enter_context(nc.sbuf_tensor("c_hi", [P, 1], f32))
    c_zero = ctx.enter_context(nc.sbuf_tensor("c_zero", [P, 1], f32))

    # --- SP: input DMA, then output DMA gated on compute completion ---
    in_dma = nc.sync.dma_start(
        xt[:], x[:].rearrange("(p k) two -> p (k two)", p=P)
    ).then_inc(in_sem, 16)
    cmp_wait = nc.sync.wait_ge(cmp_sem, 1)
    out_dma = nc.sync.dma_start(
        out[:].rearrange("(p k) two -> p (k two)", p=P), ot[:]
    ).then_inc(out_sem, 16)
    nc.sync.wait_ge(out_sem, 16)

    # --- DVE: constants + 7-op chain ---
    vector = nc.vector
    vector.memset(c_m23[:], M23)
    vector.memset(c_hi[:], HI)
    vector.memset(c_zero[:], 0.0)
    in_wait = vector.wait_ge(in_sem, 16)
    # The DVE pipeline has NO reliable same-engine RAW interlock: a
    # dependent read issued right behind a write intermittently sees stale
    # data on real HW (observed: op4 reading pre-op3 vbuf; op5's self-mult
    # reading a stale Ptr-scalar giving dfg*2^23). Every dependent stage
    # must be separated by a drain. The two final STT ops are mutually
    # independent and share one drain.
    chain0 = vector.tensor_scalar(tt_[:], xt[:], INV, C1, Alu.mult, Alu.add)
    vector.drain()
    vector.tensor_scalar(mt[:], tt_[:], c_m23[:], c_hi[:], Alu.add, Alu.min)
    vector.drain()
    vector.tensor_scalar(vt[:], mt[:], c_m23[:], c_zero[:], Alu.subtract, Alu.max)
    vector.drain()
    sq3 = tt_[:].rearrange("p (k two) -> p k two", two=2)
    o3 = ot[:].rearrange("p (k two) -> p k two", two=2)
    v3 = vt[:].rearrange("p (k two) -> p k two", two=2)
    # pos depends only on v — issue it alongside dfg (independent ops share
    # a stage, so the drain covers both). Note: plain TensorTensor /
    # TensorReduce with strided [P,K,2]-view operands mis-executes on real
    # HW even when fenced — the TensorScalarPtr scalar_tensor_tensor form
    # works (verified).
    vector.tensor_tensor(mt[:], tt_[:], vt[:], Alu.subtract)
    vector.scalar_tensor_tensor(
        o3[:, :, 1], v3[:, :, 1], float(GRID), v3[:, :, 0], Alu.mult, Alu.add
    )
    vector.drain()
    vector.tensor_tensor(tt_[:], mt[:], mt[:], Alu.mult)
    vector.drain()
    vector.scalar_tensor_tensor(
        o3[:, :, 0], sq3[:, :, 0], 1.0, sq3[:, :, 1], Alu.mult, Alu.add
    )
    # The completion signal rides a drain so all writes are committed
    # before the out-DMA reads ot.
    vector.drain().then_inc(cmp_sem, 1)

    # --- IR surgery: attach waits to their consumers, hoist the input DMA ---
    fn = nc.m.functions[0]
    bb = fn.blocks[0]
    insts = list(bb.instructions)

    def attach_wait(dst, src):
        w = list(src.ins.sync_info.on_wait)
        si = dst.ins.sync_info
        if si is None:
            dst.ins.sync_info = mybir.SyncInfo(on_wait=w, on_update=[])
        else:
            si.on_wait.extend(w)

    attach_wait(chain0, in_wait)
    attach_wait(out_dma, cmp_wait)

    dead = {in_wait.ins.name, cmp_wait.ins.name}
    insts = [i for i in insts if i.name not in dead]
    insts.remove(in_dma.ins)
    insts.insert(0, in_dma.ins)
    bb.instructions = insts

    nc.finalize()
    return nc


_CACHE = {}


def _get_program():
    if "nc" not in _CACHE:
        _CACHE["nc"] = _build_program()
    return _CACHE["nc"]


def _check_grid(protos):
    """The program hardcodes the grid; verify the input actually matches."""
    g = np.asarray(protos, dtype=np.float32)
    mids = np.arange(GRID, dtype=np.float64) * STEP + FIRST
    assert g.shape == (GRID * GRID, 2), g.shape
    assert np.array_equal(g[:GRID, 0].astype(np.float64), mids), "dim0 grid mismatch"
    assert np.array_equal(g[::GRID, 1].astype(np.float64), mids), "dim1 grid mismatch"


def kernel(x, protos):
    from concourse.bass_utils import run_bass_kernel_spmd

    x = np.ascontiguousarray(np.asarray(x, dtype=np.float32))
    _check_grid(protos)

    nc = _get_program()
    shards = np.split(x, N_CORES, axis=0)
    in_maps = [{"x": s} for s in shards]
    res = run_bass_kernel_spmd(nc, in_maps, core_ids=list(range(N_CORES)))

    d2 = np.empty(N, dtype=np.float32)
    posf = np.empty(N, dtype=np.float32)
    for c, r in enumerate(res.results):
        o = np.asarray(r["out"], dtype=np.float32).reshape(PTS, 2)
        d2[c * PTS : (c + 1) * PTS] = o[:, 0]
        posf[c * PTS : (c + 1) * PTS] = o[:, 1]

    mindist = np.sqrt(d2, dtype=np.float32) * np.float32(STEP)
    pos = posf.astype(np.int32)
    return mindist, pos
